# revision 1
# baseline (speedup 1.0000x reference)
"""Trainium2 Bass kernel for LittleBitLinearHF.

Computation (per reference):
    y = ((x * g) @ sign(V) * ell) @ sign(U).T * h + bias
with x (4, 2048, 4096) f32, U/V (4096, 128), rank r=128.

Strategy:
  * Data-parallel: shard the 8192 tokens across 8 NeuronCores (1024 each),
    params replicated (~4 MiB). No collectives.
  * Host-side prep (cheap, O(params) except the x re-layout):
      - x shard is passed TRANSPOSED (d_in, t) so the contraction dim lands
        on SBUF partitions with perfectly contiguous DMA - no on-chip
        transposes needed.
      - Vg  = g[:,None] * sign(V)            (d_in, r)    - folds input scale
      - Uf  = ell[:,None] * (sign(U)*h).T    (r, d_out)   - folds rank+output scale
      - bias replicated to (128, d_out) for a free-dim broadcast add on DVE.
  * Device per core:
      GEMM1: y1T(r, t_chunk=512) += Vg[d_tile].T @ xT[d_tile, chunk]
             accumulated over 32 d-tiles in one PSUM bank.
      GEMM2: out(t_blk=128, o_chunk=512) = y1T[:, blk].T @ Uf[:, chunk]
             then DVE adds bias while evacuating PSUM -> SBUF -> DMA out.
"""

import ml_dtypes
import numpy as np

import concourse.bass as bass
import concourse.mybir as mybir
import concourse.tile as tile
from concourse.bass_utils import run_bass_kernel_spmd

N_CORES = 8
B, S, D_IN, D_OUT, R = 4, 2048, 4096, 4096, 128
T = B * S                      # 8192 tokens
T_CORE = T // N_CORES          # 1024 tokens per core
T_CHUNK = 512                  # GEMM1 moving free dim (fp32 max)
O_CHUNK = 512                  # GEMM2 moving free dim (one PSUM bank)
P = 128
N_DT = D_IN // P               # 32 d_in tiles
DMA_GRP = 4                    # d-tiles per x DMA (1 MiB per dma_start)
F32 = mybir.dt.float32
BF16 = mybir.dt.bfloat16

_CACHED = {}


def _build_nc():
    from concourse.bacc import Bacc
    nc = Bacc()
    xh = nc.dram_tensor("xh", [D_IN, T_CORE], BF16, kind="ExternalInput")
    xl = nc.dram_tensor("xl", [D_IN, T_CORE], BF16, kind="ExternalInput")
    # vg is pre-packed on host to partition-major (p, n_dt, r) so the DMA
    # reads 8 KiB contiguous per partition (the natural (d,r) layout gives
    # 256 B chunks which crawl at ~60 GB/s and gate kernel start).
    vgh = nc.dram_tensor("vgh", [P, N_DT * R], BF16, kind="ExternalInput")
    vgl = nc.dram_tensor("vgl", [P, N_DT * R], BF16, kind="ExternalInput")
    ufh = nc.dram_tensor("ufh", [R, D_OUT], BF16, kind="ExternalInput")
    ufl = nc.dram_tensor("ufl", [R, D_OUT], BF16, kind="ExternalInput")
    bb = nc.dram_tensor("bb", [1, D_OUT], F32, kind="ExternalInput")
    y = nc.dram_tensor("y", [T_CORE, D_OUT], F32, kind="ExternalOutput")

    with tile.TileContext(nc) as tc:
        with (
            tc.tile_pool(name="params", bufs=1) as ppool,
            tc.tile_pool(name="xin", bufs=16) as xpool,
            tc.tile_pool(name="xin_lo", bufs=8) as xlpool,
            tc.tile_pool(name="y1sb", bufs=2) as y1pool,
            tc.tile_pool(name="outsb", bufs=3) as opool,
            tc.tile_pool(name="ps_y1", bufs=2, space=bass.MemorySpace.PSUM) as ps1,
            tc.tile_pool(name="ps_o", bufs=4, space=bass.MemorySpace.PSUM) as ps2,
        ):
            # ---- params on the GpSimd DMA stream: inputs own SP and
            # outputs own ACT, so the three streams never serialize ----
            vgh_sb = ppool.tile([P, N_DT, R], BF16)
            nc.gpsimd.dma_start(vgh_sb[:], vgh.rearrange("p (n r) -> p n r", n=N_DT))
            vgl_sb = ppool.tile([P, N_DT, R], BF16)
            nc.gpsimd.dma_start(vgl_sb[:], vgl.rearrange("p (n r) -> p n r", n=N_DT))
            ufh_sb = ppool.tile([P, D_OUT], BF16)
            ufl_sb = ppool.tile([P, D_OUT], BF16)
            bb_sb = ppool.tile([P, D_OUT], F32)
            uf_loaded = False

            n_chunks = T_CORE // T_CHUNK
            for c in range(n_chunks):
                t0 = c * T_CHUNK
                # ---- x DMAs: (128, DMA_GRP, 512) bf16 tiles; all hi
                # before lo so the hi-sweep can start after ~1.5 MiB ----
                xh_sb, xl_sb = [], []
                for gs in range(N_DT // DMA_GRP):
                    lo0 = gs * DMA_GRP * P
                    hi0 = (gs + 1) * DMA_GRP * P
                    th = xpool.tile([P, DMA_GRP, T_CHUNK], BF16, tag="xh")
                    nc.sync.dma_start(
                        th[:], xh[lo0:hi0, t0:t0 + T_CHUNK]
                        .rearrange("(n p) t -> p n t", p=P))
                    xh_sb.append(th)
                for gs in range(N_DT // DMA_GRP):
                    lo0 = gs * DMA_GRP * P
                    hi0 = (gs + 1) * DMA_GRP * P
                    tl = xlpool.tile([P, DMA_GRP, T_CHUNK], BF16, tag="xl")
                    nc.sync.dma_start(
                        tl[:], xl[lo0:hi0, t0:t0 + T_CHUNK]
                        .rearrange("(n p) t -> p n t", p=P))
                    xl_sb.append(tl)


                # ---- GEMM1: y1T (r, 512), 3 bf16 cross terms x 32 d-tiles --
                # (xh + xl) @ (Vgh + Vgl) ~ xh@Vgh + xh@Vgl + xl@Vgh
                # (the dropped xl@Vgl term is O(2^-18) relative)
                # Sweep-ordered: all hi*hi first (only vgh + xh needed at
                # kernel start), then the two correction sweeps.
                y1_ps = ps1.tile([R, T_CHUNK], F32)
                n_mm = 3 * N_DT
                k = 0
                for lhs, rhs in ((vgh_sb, xh_sb), (vgl_sb, xh_sb),
                                 (vgh_sb, xl_sb)):
                    for i in range(N_DT):
                        g, j = i // DMA_GRP, i % DMA_GRP
                        mm = nc.tensor.matmul(
                            y1_ps[:],
                            lhs[:, i, :],
                            rhs[g][:, j, :],
                            start=(k == 0),
                            stop=(k == n_mm - 1),
                        )
                        k += 1
                    if c == 0 and not uf_loaded:
                        # GEMM2 params, held behind GEMM1 sweep 1 so the
                        # early HBM window is pure x traffic
                        from concourse.tile_rust import add_dep_helper
                        for dst, srct in ((ufh_sb, ufh), (ufl_sb, ufl),
                                          (bb_sb[0:1, :], bb)):
                            dma = nc.gpsimd.dma_start(dst, srct[:])
                            add_dep_helper(dma.ins, mm.ins,
                                           reason="params after sweep1")
                        # bias broadcast: 16 KiB from HBM, then replicate
                        # across partitions on GpSimd (no HBM cost)
                        nc.gpsimd.partition_broadcast(bb_sb[:], bb_sb[0:1, :])
                        uf_loaded = True
                y1_sb = y1pool.tile([R, T_CHUNK], F32)
                nc.vector.tensor_copy(y1_sb[:], y1_ps[:])
                y1h_sb = y1pool.tile([R, T_CHUNK], BF16, tag="y1h")
                nc.vector.tensor_copy(y1h_sb[:], y1_sb[:])
                y1l_sb = y1pool.tile([R, T_CHUNK], BF16, tag="y1l")
                nc.vector.tensor_sub(y1l_sb[:], y1_sb[:], y1h_sb[:])

                # ---- GEMM2 + bias ----
                for tb in range(T_CHUNK // P):
                    out_sb = opool.tile([P, D_OUT], F32)
                    ts = slice(tb * P, (tb + 1) * P)
                    for oc in range(D_OUT // O_CHUNK):
                        o0 = oc * O_CHUNK
                        ps = ps2.tile([P, O_CHUNK], F32)
                        for kk, (lhs, rhs) in enumerate((
                                (y1h_sb, ufh_sb), (y1h_sb, ufl_sb),
                                (y1l_sb, ufh_sb))):
                            nc.tensor.matmul(
                                ps[:],
                                lhs[:, ts],
                                rhs[:, o0:o0 + O_CHUNK],
                                start=(kk == 0),
                                stop=(kk == 2),
                            )
                        nc.vector.tensor_add(
                            out_sb[:, o0:o0 + O_CHUNK],
                            ps[:],
                            bb_sb[:, o0:o0 + O_CHUNK],
                        )
                        if oc == 3:
                            row0 = t0 + tb * P
                            nc.scalar.dma_start(
                                y[row0:row0 + P, 0:D_OUT // 2],
                                out_sb[:, 0:D_OUT // 2])
                    row0 = t0 + tb * P
                    nc.scalar.dma_start(y[row0:row0 + P, D_OUT // 2:],
                                        out_sb[:, D_OUT // 2:])

    nc.finalize()
    return nc


def _get_nc():
    if "nc" not in _CACHED:
        _CACHED["nc"] = _build_nc()
    return _CACHED["nc"]


def _split_bf16(a):
    """Lossless-enough two-term bf16 split: a ~ hi + lo, err ~2^-18 |a|."""
    hi = a.astype(ml_dtypes.bfloat16)
    lo = (a - hi.astype(np.float32)).astype(ml_dtypes.bfloat16)
    return hi, lo


def _prep_inputs(x, U_fp, V_fp, h, g, ell, bias):
    x = np.asarray(x, dtype=np.float32).reshape(T, D_IN)
    U_fp = np.asarray(U_fp, dtype=np.float32)
    V_fp = np.asarray(V_fp, dtype=np.float32)
    h = np.asarray(h, dtype=np.float32)
    g = np.asarray(g, dtype=np.float32)
    ell = np.asarray(ell, dtype=np.float32)
    bias = np.asarray(bias, dtype=np.float32)

    U_sign = np.where(U_fp >= 0, np.float32(1.0), np.float32(-1.0))
    V_sign = np.where(V_fp >= 0, np.float32(1.0), np.float32(-1.0))
    vg_host = V_sign * g[:, None]                                 # (d_in, r)
    vgh_host, vgl_host = _split_bf16(vg_host)
    # pack (d_in, r) -> (p, n_dt*r) partition-major for contiguous DMA
    def _pack_vg(a):
        return np.ascontiguousarray(
            a.reshape(N_DT, P, R).transpose(1, 0, 2).reshape(P, N_DT * R))
    vgh_host = _pack_vg(vgh_host)
    vgl_host = _pack_vg(vgl_host)
    uf_host = ell[:, None] * (U_sign * h[:, None]).T               # (r, d_out)
    ufh_host, ufl_host = _split_bf16(uf_host)
    ufh_host = np.ascontiguousarray(ufh_host)
    ufl_host = np.ascontiguousarray(ufl_host)
    bb_host = np.ascontiguousarray(bias[None, :])

    in_maps = []
    for cidx in range(N_CORES):
        shard = x[cidx * T_CORE:(cidx + 1) * T_CORE]
        xT_c = np.ascontiguousarray(shard.T)
        xh_c, xl_c = _split_bf16(xT_c)
        in_maps.append({
            "xh": np.ascontiguousarray(xh_c),
            "xl": np.ascontiguousarray(xl_c),
            "vgh": vgh_host,
            "vgl": vgl_host,
            "ufh": ufh_host,
            "ufl": ufl_host,
            "bb": bb_host,
        })
    return in_maps


def kernel(x, U_fp, V_fp, h, g, ell, bias, _run_kwargs=None):
    in_maps = _prep_inputs(x, U_fp, V_fp, h, g, ell, bias)
    nc = _get_nc()
    kw = _run_kwargs or {}
    res = run_bass_kernel_spmd(nc, in_maps, list(range(N_CORES)), **kw)
    if _run_kwargs is not None:
        _CACHED["last_results"] = res
    out = np.concatenate([res.results[c]["y"] for c in range(N_CORES)], axis=0)
    return out.reshape(B, S, D_OUT)



# revision 8
# speedup vs baseline: 1.9005x; 1.9005x over previous
"""Trainium2 Bass kernel for LittleBitLinearHF.

Computation (per reference):
    y = ((x * g) @ sign(V) * ell) @ sign(U).T * h + bias
with x (4, 2048, 4096) f32, U/V (4096, 128), rank r=128.

Strategy (memory-roofline oriented; tolerance is rel_err < 2e-2):
  * Data-parallel: 8192 tokens over 8 cores (1024 each), params replicated.
  * Quantization plan (host-side, measured end-to-end err 1.43e-2 on the
    deterministic seed-0 inputs):
      - xq  = e3m4(x * g * 2^-k[d])   1 byte/elt  (k per d_in column keeps
              |values| <= 15.5; k==0 for this data)
      - vs  = sign(V) * 2^k[d]        e3m4, EXACT (+-pow2)
      - uf  = ell * (sign(U)*h).T     bf16 (r, d_out)
      - y1 evacuated to bf16; output written bf16, upconverted on host.
    GEMM1 runs fp8e3 at 1x rate (full-precision upcast path keeps m4).
  * Layouts are fully host-packed so every DMA is contiguous per partition:
      xq  [p, c, dt, t]   chunks c of 512 tokens, dt = d_in/128 tile
      vs  [p, dt, r]
      y   [p, c, ot, t]   ot = d_out/128 tile (host unpacks + adds nothing)
  * Device per chunk c (512 tokens):
      GEMM1: y1(r=128, 512) += vs[:,dt,:].T @ xq[:,dt,:] over 32 dt (PSUM)
      y1 -> bf16 (gpsimd)
      GEMM2: out(o=128, 512) = uf[:,ot].T @ y1  per ot; evac adds per-
             partition bias (DVE tensor_scalar / ACT Identity+bias) -> bf16
      out groups of 8 ot DMA'd out on sync/scalar queues.
    Queues: x on tensor engine queue, params on gpsimd, out on sync+scalar.
"""

import ml_dtypes
import numpy as np

import concourse.bass as bass
import concourse.mybir as mybir
import concourse.tile as tile
from concourse.bass_utils import run_bass_kernel_spmd

N_CORES = 8
B, S, D_IN, D_OUT, R = 4, 2048, 4096, 4096, 128
T = B * S                      # 8192 tokens
T_CORE = T // N_CORES          # 1024 tokens per core
T_CHUNK = 512                  # tokens per chunk (one PSUM bank of f32)
N_CHUNKS = T_CORE // T_CHUNK
P = 128
N_DT = D_IN // P               # 32 d_in tiles
N_OT = D_OUT // P              # 32 d_out tiles
X_GRP = 8                      # dt tiles per x sub-DMA (512 KiB)
O_GRP = 8                      # ot tiles per out sub-DMA (1 MiB)
F32 = mybir.dt.float32
BF16 = mybir.dt.bfloat16
FP8 = mybir.dt.float8e3

USE_FP8 = True                 # False -> bf16 x (safer, ~4.4e-3 err)
XDT = FP8 if USE_FP8 else BF16
E3M4_MAX = 15.5

_CACHED = {}

# evac engine per ot within a group of 8: V=vector, A=scalar(ACT)
# (gpsimd cannot access PSUM)
_EVAC_PATTERN = "VAVAVAVA"


def _build_nc():
    from concourse.bacc import Bacc
    nc = Bacc()
    xq = nc.dram_tensor("xq", [P, N_CHUNKS * N_DT * T_CHUNK], XDT,
                        kind="ExternalInput")
    vs = nc.dram_tensor("vs", [P, N_DT * R], XDT, kind="ExternalInput")
    uf = nc.dram_tensor("uf", [R, D_OUT], BF16, kind="ExternalInput")
    bp = nc.dram_tensor("bp", [P, N_OT], F32, kind="ExternalInput")
    y = nc.dram_tensor("y", [P, N_CHUNKS * N_OT * T_CHUNK], BF16,
                       kind="ExternalOutput")

    with tile.TileContext(nc) as tc:
        with (
            tc.tile_pool(name="params", bufs=1) as ppool,
            tc.tile_pool(name="xin", bufs=2) as xpool,
            tc.tile_pool(name="y1sb", bufs=2) as y1pool,
            tc.tile_pool(name="outsb", bufs=2) as opool,
            tc.tile_pool(name="ps_y1", bufs=2, space=bass.MemorySpace.PSUM) as ps1,
            tc.tile_pool(name="ps_o", bufs=4, space=bass.MemorySpace.PSUM) as ps2,
        ):
            # ---- params on gpsimd queue ----
            vs_sb = ppool.tile([P, N_DT, R], XDT)
            nc.gpsimd.dma_start(vs_sb[:], vs.rearrange("p (n r) -> p n r",
                                                       n=N_DT))
            uf_sb = ppool.tile([R, D_OUT], BF16)
            nc.gpsimd.dma_start(uf_sb[:], uf[:])
            bp_sb = ppool.tile([P, N_OT], F32)
            nc.gpsimd.dma_start(bp_sb[:], bp[:])

            # ---- all x sub-DMAs up front on the tensor-engine queue ----
            x_sb = []
            for c in range(N_CHUNKS):
                xt = xpool.tile([P, N_DT, T_CHUNK], XDT, tag="x")
                x_sb.append(xt)
                for gs in range(N_DT // X_GRP):
                    lo = c * N_DT * T_CHUNK + gs * X_GRP * T_CHUNK
                    hi = lo + X_GRP * T_CHUNK
                    nc.sync.dma_start(
                        xt[:, gs * X_GRP:(gs + 1) * X_GRP, :],
                        xq[:, lo:hi].rearrange("p (n t) -> p n t", n=X_GRP))

            for c in range(N_CHUNKS):
                # ---- GEMM1: y1 (r, T_CHUNK) over 32 dt tiles ----
                y1_ps = ps1.tile([R, T_CHUNK], F32)
                for dt in range(N_DT):
                    nc.tensor.matmul(
                        y1_ps[:],
                        vs_sb[:, dt, :],
                        x_sb[c][:, dt, :],
                        start=(dt == 0),
                        stop=(dt == N_DT - 1),
                    )
                y1_sb = y1pool.tile([R, T_CHUNK], BF16)
                nc.vector.tensor_copy(y1_sb[:], y1_ps[:])

                # ---- GEMM2 (transposed: partitions = d_out tile) ----
                out_sb = opool.tile([P, N_OT, T_CHUNK], BF16)
                for ot in range(N_OT):
                    ps = ps2.tile([P, T_CHUNK], F32)
                    nc.tensor.matmul(
                        ps[:],
                        uf_sb[:, ot * P:(ot + 1) * P],
                        y1_sb[:],
                        start=True,
                        stop=True,
                    )
                    ev = _EVAC_PATTERN[ot % len(_EVAC_PATTERN)]
                    if ev == "A":
                        nc.scalar.activation(
                            out_sb[:, ot, :], ps[:],
                            mybir.ActivationFunctionType.Identity,
                            bias=bp_sb[:, ot:ot + 1])
                    else:
                        nc.vector.tensor_scalar_add(
                            out_sb[:, ot, :], ps[:], bp_sb[:, ot:ot + 1])
                    if ot % O_GRP == O_GRP - 1:
                        g0 = ot - (O_GRP - 1)
                        lo = c * N_OT * T_CHUNK + g0 * T_CHUNK
                        hi = lo + O_GRP * T_CHUNK
                        dq = nc.scalar if (ot // O_GRP) % 2 == 0 else nc.gpsimd
                        dq.dma_start(
                            y[:, lo:hi].rearrange("p (n t) -> p n t",
                                                  n=O_GRP),
                            out_sb[:, g0:ot + 1, :])

    nc.finalize()
    return nc


def _get_nc():
    if "nc" not in _CACHED:
        _CACHED["nc"] = _build_nc()
    return _CACHED["nc"]


def _prep_inputs(x, U_fp, V_fp, h, g, ell, bias):
    x = np.asarray(x, dtype=np.float32).reshape(T, D_IN)
    U_fp = np.asarray(U_fp, dtype=np.float32)
    V_fp = np.asarray(V_fp, dtype=np.float32)
    h = np.asarray(h, dtype=np.float32)
    g = np.asarray(g, dtype=np.float32)
    ell = np.asarray(ell, dtype=np.float32)
    bias = np.asarray(bias, dtype=np.float32)

    U_sign = np.where(U_fp >= 0, np.float32(1.0), np.float32(-1.0))
    V_sign = np.where(V_fp >= 0, np.float32(1.0), np.float32(-1.0))

    np_xdt = mybir.dt.np(XDT)
    if USE_FP8:
        xg = x * g[None, :]
        # per-column power-of-two scale so |xq| <= 15.5 (exact inverse on vs)
        mx = np.abs(xg).max(axis=0)
        k = np.maximum(0, np.ceil(np.log2(np.maximum(mx, 1e-30) / E3M4_MAX)))
        k = k.astype(np.float32)
        assert k.max() <= 3.0, "pow2 scale exceeds e3m4 range"
        scale = (2.0 ** k).astype(np.float32)
        xh = np.clip(xg / scale[None, :], -E3M4_MAX, E3M4_MAX).astype(np_xdt)
        vs_host = (V_sign * scale[:, None]).astype(np_xdt)
    else:
        xh = x.astype(np_xdt)
        vs_host = (V_sign * g[:, None]).astype(np_xdt)

    # pack vs (d_in, r) -> (p, dt*r)
    vs_host = np.ascontiguousarray(
        vs_host.reshape(N_DT, P, R).transpose(1, 0, 2).reshape(P, N_DT * R))
    uf_host = np.ascontiguousarray(
        (ell[:, None] * (U_sign * h[:, None]).T).astype(ml_dtypes.bfloat16))
    bp_host = np.ascontiguousarray(bias.reshape(N_OT, P).T)

    in_maps = []
    for cidx in range(N_CORES):
        shard = xh[cidx * T_CORE:(cidx + 1) * T_CORE]      # (1024, 4096)
        xp = shard.reshape(N_CHUNKS, T_CHUNK, N_DT, P)
        xp = np.ascontiguousarray(
            xp.transpose(3, 0, 2, 1).reshape(P, N_CHUNKS * N_DT * T_CHUNK))
        in_maps.append({
            "xq": xp,
            "vs": vs_host,
            "uf": uf_host,
            "bp": bp_host,
        })
    return in_maps


def _unpack_core(yp):
    """(P, N_CHUNKS*N_OT*T_CHUNK) packed bf16 -> (T_CORE, D_OUT) f32."""
    yp = np.asarray(yp).reshape(P, N_CHUNKS, N_OT, T_CHUNK)
    return yp.transpose(1, 3, 2, 0).reshape(T_CORE, D_OUT).astype(np.float32)


def _unpack_output(res):
    outs = [_unpack_core(res.results[c]["y"]) for c in range(N_CORES)]
    return np.concatenate(outs, axis=0).reshape(B, S, D_OUT)


def kernel(x, U_fp, V_fp, h, g, ell, bias, _run_kwargs=None):
    in_maps = _prep_inputs(x, U_fp, V_fp, h, g, ell, bias)
    nc = _get_nc()
    kw = _run_kwargs or {}
    res = run_bass_kernel_spmd(nc, in_maps, list(range(N_CORES)), **kw)
    if _run_kwargs is not None:
        _CACHED["last_results"] = res
    return _unpack_output(res)
